# revision 19
# baseline (speedup 1.0000x reference)
"""Trainium2 Bass kernel for nn_FFTConv:
  out[b,p,h,w] = sum_c z[b,c,h,w]*filt[c,p,h,w] + sum_c bias[c,p,h,w]
with complex64 z[8,32,128,128], filt/bias[32,32,128,128].

Strategy
--------
Shard the spatial H dim across the 8 NeuronCores (16 rows each) -- pure data
parallelism, zero replication, no collectives.

Each output pixel needs a tiny complex matmul out[p,b] = F(px)^T @ z(px) with
K=c=32, M=p=32, N=b=8. Complex is expanded to a K=64 real contraction per
pixel ([fr;fi] rows against [zr|zi; -zi|zr] rhs), and TWO pixels are packed
into each K=128 matmul with a block-diagonal rhs:

  lhsT (128 x 32p) rows = [px0: fr(32c); fi(32c) | px1: fr; fi]
  rhs  (128 x 32)  cols = (e2, u2, b8):
      px0 cols (0-15):  rows 0-31 [zr|zi], rows 32-63 [-zi|zr], rows 64-127 = 0
      px1 cols (16-31): rows 0-63 = 0, rows 64-95 [zr|zi], rows 96-127 [-zi|zr]
  -> one matmul per pixel pair (1024 per core). This matters beyond FLOPs:
  the PE's per-matmul completion semaphore increments drain at ~85ns each and
  pace the whole kernel, so fewer/bigger matmuls win.

The bias-channel reduction sum_c bias[c,p,px] runs on the otherwise-idle DVE
(reduce over c, then added during psum evacuation), not as extra PE work.

Pairs are packed 4-at-a-time across the PE's four 32-col sub-array groups
(tile_position=(0, 32j)). The rhs zero blocks are persistent in SBUF (memset
once); only real z data is DMA'd.

Engine split: input DMAs on the SP HWDGE ring, output DMAs on the ACT ring
(whose compute-completion waits must not head-of-line-block input prefetch),
bias reduce + psum evacuation on DVE.

Host-side numpy only reorders data (transpose/interleave/negate) into
pixel-major DMA-friendly layouts; all arithmetic happens on device.

Layouts (per core, dtype DT = float32):
  local pixel px in [0,2048); sb = px//128; within sb: s = (px%128)//8,
  j = (px%8)//2 (col slot), e = px%2 (pair member); pair pr = s*4 + j.
  wc [sb,128,2048]: row = e*64 + d*32 + c (d: fr,fi); free = pr*32 + p
  zd [sb,2(e),64,1024]: row = d*32 + c; free = pr*16 + u*8 + b
      d0: [u0=zr, u1=zi] ; d1: [u0=-zi, u1=zr]
  bias_t [sb,128,2048]: part q = 32j + p; free = s*128 + e*64 + u*32 + c
  out_dev [sb,128,512] f32: part q = 32j + p; free = s*32 + e*16 + u*8 + b
"""

import numpy as np

B, C, P, H, W = 8, 32, 32, 128, 128
NCORES = 8
HPER = H // NCORES          # 16
PX = HPER * W               # 2048
NSB = 16                    # super-batches per core
NS = 16                     # pair-serial slots per sb
ZFREE = 2048                # zb cols per sb (64 pairs x 32)

_DT_NP = np.float32
_DT_BASS = "float32"


def _pixelize(arr_t, n):
    """arr_t: [h, w, ...]; returns [sb, s, j, e, ...] for core n's h-strip."""
    a = arr_t[HPER * n:HPER * (n + 1)]
    a = a.reshape(PX, *a.shape[2:])
    return a.reshape(NSB, NS, 4, 2, *a.shape[1:])


def _prepare_inputs(z, filt, bias):
    zr = np.ascontiguousarray(z.real).astype(_DT_NP)
    zi = np.ascontiguousarray(z.imag).astype(_DT_NP)
    fr = np.ascontiguousarray(filt.real).astype(_DT_NP)
    fi = np.ascontiguousarray(filt.imag).astype(_DT_NP)
    br = np.ascontiguousarray(bias.real).astype(_DT_NP)
    bi = np.ascontiguousarray(bias.imag).astype(_DT_NP)

    fr_t = fr.transpose(2, 3, 0, 1)   # [h, w, c, p]
    fi_t = fi.transpose(2, 3, 0, 1)
    br_t = br.transpose(2, 3, 0, 1)
    bi_t = bi.transpose(2, 3, 0, 1)
    zr_t = zr.transpose(2, 3, 1, 0)   # [h, w, c, b]
    zi_t = zi.transpose(2, 3, 1, 0)

    in_maps = []
    for n in range(NCORES):
        frp = _pixelize(fr_t, n)      # [sb, s, j, e, c, p]
        fip = _pixelize(fi_t, n)
        # wc rows = (e, d, c): [sb, s, j, e, d, c, p] -> [sb, e, d, c, s, j, p]
        X = np.stack([frp, fip], axis=4)
        wc = X.transpose(0, 3, 4, 5, 1, 2, 6).reshape(NSB, 128, 2048)

        zrp = _pixelize(zr_t, n)      # [sb, s, j, e, c, b]
        zip_ = _pixelize(zi_t, n)
        # zd [sb, e, d, c, s, j, u, b]
        zd = np.empty((NSB, NS, 4, 2, 2, 32, 2, 8), _DT_NP)
        zd[..., 0, :, 0, :] = zrp
        zd[..., 0, :, 1, :] = zip_
        zd[..., 1, :, 0, :] = -zip_
        zd[..., 1, :, 1, :] = zrp
        # [sb, s, j, e, d, c, u, b] -> [sb, e, d, c, s, j, u, b]
        zd = zd.transpose(0, 3, 4, 5, 1, 2, 6, 7).reshape(NSB, 2, 64, 1024)

        brp = _pixelize(br_t, n)      # [sb, s, j, e, c, p]
        bip = _pixelize(bi_t, n)
        Y = np.stack([brp, bip], axis=4)   # [sb, s, j, e, u, c, p]
        # -> [sb, j, p, s, e, u, c]
        bt = Y.transpose(0, 2, 6, 1, 3, 4, 5).reshape(NSB, 128, 2048)

        in_maps.append({
            "wc": np.ascontiguousarray(wc),
            "zd": np.ascontiguousarray(zd),
            "bt": np.ascontiguousarray(bt),
        })
    return in_maps


def _assemble_output(res_list):
    out = np.empty((B, P, H, W), np.complex64)
    for n in range(NCORES):
        # part q = 32j + p ; free f = s*32 + e*16 + u*8 + b
        arr = res_list[n].reshape(NSB, 4, 32, NS, 2, 2, 8)
        # [sb, j, p, s, e, u, b] -> [b, p, sb, s, j, e, u]
        arr = arr.transpose(6, 2, 0, 3, 1, 4, 5)
        arr = np.ascontiguousarray(arr).reshape(B, P, HPER, W, 2)
        out[:, :, HPER * n:HPER * (n + 1), :] = arr[..., 0] + 1j * arr[..., 1]
    return out


_NC_CACHE = [None]


def _build_bass():
    if _NC_CACHE[0] is not None:
        return _NC_CACHE[0]
    import concourse.mybir as mybir
    import concourse.tile as tile
    from concourse import bacc

    f32 = mybir.dt.float32
    dt = getattr(mybir.dt, _DT_BASS)
    nc = bacc.Bacc("TRN2", target_bir_lowering=False, debug=False)
    wc_d = nc.dram_tensor("wc", [NSB, 128, 2048], dt, kind="ExternalInput")
    zd_d = nc.dram_tensor("zd", [NSB, 2, 64, 1024], dt, kind="ExternalInput")
    bt_d = nc.dram_tensor("bt", [NSB, 128, 2048], dt, kind="ExternalInput")
    out_d = nc.dram_tensor("out_dev", [NSB, 128, 512], f32,
                           kind="ExternalOutput")

    # Persistent triple-buffered z tensors (fixed SBUF allocations, outside
    # the Tile pools so their slots can't be recycled).
    zb_handles = [
        nc.alloc_sbuf_tensor(f"zb{i}", [128, ZFREE], dt) for i in range(3)
    ]

    with tile.TileContext(nc) as tc:
        with (
            tc.tile_pool(name="wp", bufs=6) as wpool,
            tc.tile_pool(name="bp", bufs=6) as bpool,
            tc.tile_pool(name="op", bufs=4) as opool,
            tc.tile_pool(name="pp", bufs=8, space="PSUM") as pspool,
        ):
            # block-diagonal rhs zero blocks, written once per z buffer.
            # zb free layout: [0:1024) = px0 data (rows 0-63) / zeros
            # (rows 64-127); [1024:2048) = zeros (rows 0-63) / px1 data
            # (rows 64-127). Keeps both the DMAs and the zero regions
            # contiguous; the matmul rhs uses a 2-level strided free AP.
            zbufs = []
            for i in range(3):
                zb = zb_handles[i][:]
                nc.vector.memset(zb[0:64, 1024:2048], 0.0)
                nc.vector.memset(zb[64:128, 0:1024], 0.0)
                zbufs.append(zb)

            for sb in range(NSB):
                wc_t = wpool.tile([128, 2048], dt, name="wc_t", tag="wc_t")
                nc.sync.dma_start(out=wc_t, in_=wc_d[sb])
                zb = zbufs[sb % 3]
                nc.sync.dma_start(out=zb[0:64, 0:1024], in_=zd_d[sb, 0])
                nc.sync.dma_start(out=zb[64:128, 1024:2048],
                                  in_=zd_d[sb, 1])
                zev = zb.rearrange("r (e f) -> r e f", e=2)
                bt_t = bpool.tile([128, 2048], dt, name="bt_t", tag="bt_t")
                nc.sync.dma_start(out=bt_t, in_=bt_d[sb])

                # bias-channel reduction on DVE: [128,(s,e,u),c32] -> [128,64]
                bs_t = bpool.tile([128, 64], f32, name="bs_t", tag="bs_t")
                nc.vector.reduce_sum(
                    bs_t, bt_t.rearrange("q (f c) -> q f c", c=32),
                    axis=mybir.AxisListType.X)

                # Full-bank psum tile (2048B/partition) so accumulation
                # zero-regions align exactly with 32-partition col-group
                # slices.
                ps = pspool.tile([128, 512], f32, name="ps", tag="ps")
                for s in range(NS):
                    # one K=128 matmul per pixel pair; consecutive matmuls
                    # hit different col-groups and run concurrently
                    for j in range(4):
                        pr = s * 4 + j
                        lhs = wc_t[:, pr * 32:pr * 32 + 32]
                        rhs = zev[:, :, pr * 16:pr * 16 + 16]
                        po = ps[32 * j:32 * j + 32, s * 32:s * 32 + 32]
                        nc.tensor.matmul(
                            po, lhs, rhs, start=True, stop=True,
                            tile_position=(0, 32 * j),
                            # sim's global group-check shadow mis-addresses
                            # partition-sliced psum outputs; the per-tensor
                            # pending-zero numerics are still modeled
                            # faithfully
                            skip_group_check=True,
                        )

                # evacuate psum + add bias in one pass per b-column
                o_t = opool.tile([128, 512], f32, name="o_t", tag="o_t")
                ov = o_t.rearrange("q (f b) -> q f b", b=8)
                pv = ps.rearrange("q (f b) -> q f b", b=8)
                for b8 in range(8):
                    nc.vector.tensor_add(ov[:, :, b8], pv[:, :, b8], bs_t)
                # output DMA on the ACT HWDGE ring: its compute-completion
                # wait must not head-of-line-block the input DMAs queued on
                # the SP ring
                nc.scalar.dma_start(out=out_d[sb], in_=o_t)

    nc.compile()
    _NC_CACHE[0] = nc
    return nc


def run(z, filt, bias, trace=False, trace_kwargs=None):
    """Returns (out, BassKernelResults)."""
    from concourse.bass_utils import run_bass_kernel_spmd
    in_maps = _prepare_inputs(z, filt, bias)
    nc = _build_bass()
    bkr = run_bass_kernel_spmd(
        nc, in_maps, core_ids=list(range(NCORES)),
        trace=trace, **(trace_kwargs or {}),
    )
    out = _assemble_output([r["out_dev"] for r in bkr.results])
    return out, bkr


def kernel(z, filt, bias):
    out, _ = run(np.asarray(z), np.asarray(filt), np.asarray(bias))
    return out


# revision 20
# speedup vs baseline: 1.0026x; 1.0026x over previous
"""Trainium2 Bass kernel for nn_FFTConv:
  out[b,p,h,w] = sum_c z[b,c,h,w]*filt[c,p,h,w] + sum_c bias[c,p,h,w]
with complex64 z[8,32,128,128], filt/bias[32,32,128,128].

Strategy
--------
Shard the spatial H dim across the 8 NeuronCores (16 rows each) -- pure data
parallelism, zero replication, no collectives.

Each output pixel needs a tiny complex matmul out[p,b] = F(px)^T @ z(px) with
K=c=32, M=p=32, N=b=8. Complex is expanded to a K=64 real contraction per
pixel ([fr;fi] rows against [zr|zi; -zi|zr] rhs), and TWO pixels are packed
into each K=128 matmul with a block-diagonal rhs:

  lhsT (128 x 32p) rows = [px0: fr(32c); fi(32c) | px1: fr; fi]
  rhs  (128 x 32)  cols = (e2, u2, b8):
      px0 cols (0-15):  rows 0-31 [zr|zi], rows 32-63 [-zi|zr], rows 64-127 = 0
      px1 cols (16-31): rows 0-63 = 0, rows 64-95 [zr|zi], rows 96-127 [-zi|zr]
  -> one matmul per pixel pair (1024 per core). This matters beyond FLOPs:
  the PE's per-matmul completion semaphore increments drain at ~85ns each and
  pace the whole kernel, so fewer/bigger matmuls win.

The bias-channel reduction sum_c bias[c,p,px] runs on the otherwise-idle DVE
(reduce over c, then added during psum evacuation), not as extra PE work.

Pairs are packed 4-at-a-time across the PE's four 32-col sub-array groups
(tile_position=(0, 32j)). The rhs zero blocks are persistent in SBUF (memset
once); only real z data is DMA'd.

Engine split: input DMAs on the SP HWDGE ring, output DMAs on the ACT ring
(whose compute-completion waits must not head-of-line-block input prefetch),
bias reduce + psum evacuation on DVE.

Host-side numpy only reorders data (transpose/interleave/negate) into
pixel-major DMA-friendly layouts; all arithmetic happens on device.

Layouts (per core, dtype DT = float32):
  local pixel px in [0,2048); sb = px//128; within sb: s = (px%128)//8,
  j = (px%8)//2 (col slot), e = px%2 (pair member); pair pr = s*4 + j.
  wc [sb,128,2048]: row = e*64 + d*32 + c (d: fr,fi); free = pr*32 + p
  zd [sb,2(e),64,1024]: row = d*32 + c; free = pr*16 + u*8 + b
      d0: [u0=zr, u1=zi] ; d1: [u0=-zi, u1=zr]
  bias_t [sb,128,2048]: part q = 32j + p; free = s*128 + e*64 + u*32 + c
  out_dev [sb,128,512] f32: part q = 32j + p; free = s*32 + e*16 + u*8 + b
"""

import numpy as np

B, C, P, H, W = 8, 32, 32, 128, 128
NCORES = 8
HPER = H // NCORES          # 16
PX = HPER * W               # 2048
NSB = 16                    # super-batches per core
NS = 16                     # pair-serial slots per sb
ZFREE = 2048                # zb cols per sb (64 pairs x 32)

_DT_NP = np.float32
_DT_BASS = "float32"


def _pixelize(arr_t, n):
    """arr_t: [h, w, ...]; returns [sb, s, j, e, ...] for core n's h-strip."""
    a = arr_t[HPER * n:HPER * (n + 1)]
    a = a.reshape(PX, *a.shape[2:])
    return a.reshape(NSB, NS, 4, 2, *a.shape[1:])


def _prepare_inputs(z, filt, bias):
    zr = np.ascontiguousarray(z.real).astype(_DT_NP)
    zi = np.ascontiguousarray(z.imag).astype(_DT_NP)
    fr = np.ascontiguousarray(filt.real).astype(_DT_NP)
    fi = np.ascontiguousarray(filt.imag).astype(_DT_NP)
    br = np.ascontiguousarray(bias.real).astype(_DT_NP)
    bi = np.ascontiguousarray(bias.imag).astype(_DT_NP)

    fr_t = fr.transpose(2, 3, 0, 1)   # [h, w, c, p]
    fi_t = fi.transpose(2, 3, 0, 1)
    br_t = br.transpose(2, 3, 0, 1)
    bi_t = bi.transpose(2, 3, 0, 1)
    zr_t = zr.transpose(2, 3, 1, 0)   # [h, w, c, b]
    zi_t = zi.transpose(2, 3, 1, 0)

    in_maps = []
    for n in range(NCORES):
        frp = _pixelize(fr_t, n)      # [sb, s, j, e, c, p]
        fip = _pixelize(fi_t, n)
        # wc rows = (e, d, c): [sb, s, j, e, d, c, p] -> [sb, e, d, c, s, j, p]
        X = np.stack([frp, fip], axis=4)
        wc = X.transpose(0, 3, 4, 5, 1, 2, 6).reshape(NSB, 128, 2048)

        zrp = _pixelize(zr_t, n)      # [sb, s, j, e, c, b]
        zip_ = _pixelize(zi_t, n)
        # zd [sb, e, d, c, s, j, u, b]
        zd = np.empty((NSB, NS, 4, 2, 2, 32, 2, 8), _DT_NP)
        zd[..., 0, :, 0, :] = zrp
        zd[..., 0, :, 1, :] = zip_
        zd[..., 1, :, 0, :] = -zip_
        zd[..., 1, :, 1, :] = zrp
        # [sb, s, j, e, d, c, u, b] -> [sb, e, d, c, s, j, u, b]
        zd = zd.transpose(0, 3, 4, 5, 1, 2, 6, 7).reshape(NSB, 2, 64, 1024)

        brp = _pixelize(br_t, n)      # [sb, s, j, e, c, p]
        bip = _pixelize(bi_t, n)
        Y = np.stack([brp, bip], axis=4)   # [sb, s, j, e, u, c, p]
        # -> [sb, j, p, s, e, u, c]
        bt = Y.transpose(0, 2, 6, 1, 3, 4, 5).reshape(NSB, 128, 2048)

        in_maps.append({
            "wc": np.ascontiguousarray(wc),
            "zd": np.ascontiguousarray(zd),
            "bt": np.ascontiguousarray(bt),
        })
    return in_maps


def _assemble_output(res_list):
    out = np.empty((B, P, H, W), np.complex64)
    for n in range(NCORES):
        # part q = 32j + p ; free f = s*32 + e*16 + u*8 + b
        arr = res_list[n].reshape(NSB, 4, 32, NS, 2, 2, 8)
        # [sb, j, p, s, e, u, b] -> [b, p, sb, s, j, e, u]
        arr = arr.transpose(6, 2, 0, 3, 1, 4, 5)
        arr = np.ascontiguousarray(arr).reshape(B, P, HPER, W, 2)
        out[:, :, HPER * n:HPER * (n + 1), :] = arr[..., 0] + 1j * arr[..., 1]
    return out


_NC_CACHE = [None]


def _build_bass():
    if _NC_CACHE[0] is not None:
        return _NC_CACHE[0]
    import concourse.mybir as mybir
    import concourse.tile as tile
    from concourse import bacc

    f32 = mybir.dt.float32
    dt = getattr(mybir.dt, _DT_BASS)
    nc = bacc.Bacc("TRN2", target_bir_lowering=False, debug=False)
    wc_d = nc.dram_tensor("wc", [NSB, 128, 2048], dt, kind="ExternalInput")
    zd_d = nc.dram_tensor("zd", [NSB, 2, 64, 1024], dt, kind="ExternalInput")
    bt_d = nc.dram_tensor("bt", [NSB, 128, 2048], dt, kind="ExternalInput")
    out_d = nc.dram_tensor("out_dev", [NSB, 128, 512], f32,
                           kind="ExternalOutput")

    # Persistent triple-buffered z tensors (fixed SBUF allocations, outside
    # the Tile pools so their slots can't be recycled).
    zb_handles = [
        nc.alloc_sbuf_tensor(f"zb{i}", [128, ZFREE], dt) for i in range(3)
    ]

    with tile.TileContext(nc) as tc:
        with (
            tc.tile_pool(name="wp", bufs=4) as wpool,
            tc.tile_pool(name="bp", bufs=4) as bpool,
            tc.tile_pool(name="op", bufs=4) as opool,
            tc.tile_pool(name="pp", bufs=8, space="PSUM") as pspool,
        ):
            # block-diagonal rhs zero blocks, written once per z buffer.
            # zb free layout: [0:1024) = px0 data (rows 0-63) / zeros
            # (rows 64-127); [1024:2048) = zeros (rows 0-63) / px1 data
            # (rows 64-127). Keeps both the DMAs and the zero regions
            # contiguous; the matmul rhs uses a 2-level strided free AP.
            zbufs = []
            for i in range(3):
                zb = zb_handles[i][:]
                nc.vector.memset(zb[0:64, 1024:2048], 0.0)
                nc.vector.memset(zb[64:128, 0:1024], 0.0)
                zbufs.append(zb)

            for sb in range(NSB):
                wc_t = wpool.tile([128, 2048], dt, name="wc_t", tag="wc_t")
                nc.sync.dma_start(out=wc_t, in_=wc_d[sb])
                zb = zbufs[sb % 3]
                nc.sync.dma_start(out=zb[0:64, 0:1024], in_=zd_d[sb, 0])
                nc.sync.dma_start(out=zb[64:128, 1024:2048],
                                  in_=zd_d[sb, 1])
                zev = zb.rearrange("r (e f) -> r e f", e=2)
                bt_t = bpool.tile([128, 2048], dt, name="bt_t", tag="bt_t")
                nc.sync.dma_start(out=bt_t, in_=bt_d[sb])

                # bias-channel reduction on DVE: [128,(s,e,u),c32] -> [128,64]
                bs_t = bpool.tile([128, 64], f32, name="bs_t", tag="bs_t")
                nc.vector.reduce_sum(
                    bs_t, bt_t.rearrange("q (f c) -> q f c", c=32),
                    axis=mybir.AxisListType.X)

                # Full-bank psum tile (2048B/partition) so accumulation
                # zero-regions align exactly with 32-partition col-group
                # slices.
                ps = pspool.tile([128, 512], f32, name="ps", tag="ps")
                for s in range(NS):
                    # one K=128 matmul per pixel pair; consecutive matmuls
                    # hit different col-groups and run concurrently
                    for j in range(4):
                        pr = s * 4 + j
                        lhs = wc_t[:, pr * 32:pr * 32 + 32]
                        rhs = zev[:, :, pr * 16:pr * 16 + 16]
                        po = ps[32 * j:32 * j + 32, s * 32:s * 32 + 32]
                        nc.tensor.matmul(
                            po, lhs, rhs, start=True, stop=True,
                            tile_position=(0, 32 * j),
                            # sim's global group-check shadow mis-addresses
                            # partition-sliced psum outputs; the per-tensor
                            # pending-zero numerics are still modeled
                            # faithfully
                            skip_group_check=True,
                        )

                # evacuate psum + add bias in one pass per b-column
                o_t = opool.tile([128, 512], f32, name="o_t", tag="o_t")
                ov = o_t.rearrange("q (f b) -> q f b", b=8)
                pv = ps.rearrange("q (f b) -> q f b", b=8)
                for b8 in range(8):
                    nc.vector.tensor_add(ov[:, :, b8], pv[:, :, b8], bs_t)
                # output DMA on the ACT HWDGE ring: its compute-completion
                # wait must not head-of-line-block the input DMAs queued on
                # the SP ring
                nc.scalar.dma_start(out=out_d[sb], in_=o_t)

    nc.compile()
    _NC_CACHE[0] = nc
    return nc


def run(z, filt, bias, trace=False, trace_kwargs=None):
    """Returns (out, BassKernelResults)."""
    from concourse.bass_utils import run_bass_kernel_spmd
    in_maps = _prepare_inputs(z, filt, bias)
    nc = _build_bass()
    bkr = run_bass_kernel_spmd(
        nc, in_maps, core_ids=list(range(NCORES)),
        trace=trace, **(trace_kwargs or {}),
    )
    out = _assemble_output([r["out_dev"] for r in bkr.results])
    return out, bkr


def kernel(z, filt, bias):
    out, _ = run(np.asarray(z), np.asarray(filt), np.asarray(bias))
    return out


# revision 21
# speedup vs baseline: 1.6249x; 1.6207x over previous
"""Trainium2 Bass kernel for nn_FFTConv:
  out[b,p,h,w] = sum_c z[b,c,h,w]*filt[c,p,h,w] + sum_c bias[c,p,h,w]
with complex64 z[8,32,128,128], filt/bias[32,32,128,128].

Strategy
--------
Shard the spatial H dim across the 8 NeuronCores (16 rows each) -- pure data
parallelism, zero replication, no collectives.

Each output pixel needs a tiny complex matmul out[p,b] = F(px)^T @ z(px) with
K=c=32, M=p=32, N=b=8. Complex is expanded to a K=64 real contraction per
pixel ([fr;fi] rows against [zr|zi; -zi|zr] rhs), and TWO pixels are packed
into each K=128 matmul with a block-diagonal rhs:

  lhsT (128 x 32p) rows = [px0: fr(32c); fi(32c) | px1: fr; fi]
  rhs  (128 x 32)  cols = (e2, u2, b8):
      px0 cols (0-15):  rows 0-31 [zr|zi], rows 32-63 [-zi|zr], rows 64-127 = 0
      px1 cols (16-31): rows 0-63 = 0, rows 64-95 [zr|zi], rows 96-127 [-zi|zr]
  -> one matmul per pixel pair (1024 per core). This matters beyond FLOPs:
  the PE's per-matmul completion semaphore increments drain at ~85ns each and
  pace the whole kernel, so fewer/bigger matmuls win.

The bias-channel reduction sum_c bias[c,p,px] runs on the otherwise-idle DVE
(reduce over c, then added during psum evacuation), not as extra PE work.

Pairs are packed 4-at-a-time across the PE's four 32-col sub-array groups
(tile_position=(0, 32j)). The rhs zero blocks are persistent in SBUF (memset
once); only real z data is DMA'd.

Engine split: input DMAs on the SP HWDGE ring, output DMAs on the ACT ring
(whose compute-completion waits must not head-of-line-block input prefetch),
bias reduce + psum evacuation on DVE.

Host-side numpy only reorders data (transpose/interleave/negate) into
pixel-major DMA-friendly layouts; all arithmetic happens on device.

Layouts (per core, dtype DT = float32):
  local pixel px in [0,2048); sb = px//128; within sb: s = (px%128)//8,
  j = (px%8)//2 (col slot), e = px%2 (pair member); pair pr = s*4 + j.
  wc [sb,128,2048]: row = e*64 + d*32 + c (d: fr,fi); free = pr*32 + p
  zd [sb,2(e),64,1024]: row = d*32 + c; free = pr*16 + u*8 + b
      d0: [u0=zr, u1=zi] ; d1: [u0=-zi, u1=zr]
  bias_t [sb,128,2048]: part q = 32j + p; free = s*128 + e*64 + u*32 + c
  out_dev [sb,128,512] f32: part q = 32j + p; free = s*32 + e*16 + u*8 + b
"""

import numpy as np

B, C, P, H, W = 8, 32, 32, 128, 128
NCORES = 8
HPER = H // NCORES          # 16
PX = HPER * W               # 2048
NSB = 16                    # super-batches per core
NS = 16                     # pair-serial slots per sb
ZFREE = 2048                # zb cols per sb (64 pairs x 32)

_DT_NP = np.float16
_DT_BASS = "float16"


def _pixelize(arr_t, n):
    """arr_t: [h, w, ...]; returns [sb, s, j, e, ...] for core n's h-strip."""
    a = arr_t[HPER * n:HPER * (n + 1)]
    a = a.reshape(PX, *a.shape[2:])
    return a.reshape(NSB, NS, 4, 2, *a.shape[1:])


def _prepare_inputs(z, filt, bias):
    zr = np.ascontiguousarray(z.real).astype(_DT_NP)
    zi = np.ascontiguousarray(z.imag).astype(_DT_NP)
    fr = np.ascontiguousarray(filt.real).astype(_DT_NP)
    fi = np.ascontiguousarray(filt.imag).astype(_DT_NP)
    br = np.ascontiguousarray(bias.real).astype(_DT_NP)
    bi = np.ascontiguousarray(bias.imag).astype(_DT_NP)

    fr_t = fr.transpose(2, 3, 0, 1)   # [h, w, c, p]
    fi_t = fi.transpose(2, 3, 0, 1)
    br_t = br.transpose(2, 3, 0, 1)
    bi_t = bi.transpose(2, 3, 0, 1)
    zr_t = zr.transpose(2, 3, 1, 0)   # [h, w, c, b]
    zi_t = zi.transpose(2, 3, 1, 0)

    in_maps = []
    for n in range(NCORES):
        frp = _pixelize(fr_t, n)      # [sb, s, j, e, c, p]
        fip = _pixelize(fi_t, n)
        # wc rows = (e, d, c): [sb, s, j, e, d, c, p] -> [sb, e, d, c, s, j, p]
        X = np.stack([frp, fip], axis=4)
        wc = X.transpose(0, 3, 4, 5, 1, 2, 6).reshape(NSB, 128, 2048)

        zrp = _pixelize(zr_t, n)      # [sb, s, j, e, c, b]
        zip_ = _pixelize(zi_t, n)
        # zd [sb, e, d, c, s, j, u, b]
        zd = np.empty((NSB, NS, 4, 2, 2, 32, 2, 8), _DT_NP)
        zd[..., 0, :, 0, :] = zrp
        zd[..., 0, :, 1, :] = zip_
        zd[..., 1, :, 0, :] = -zip_
        zd[..., 1, :, 1, :] = zrp
        # [sb, s, j, e, d, c, u, b] -> [sb, e, d, c, s, j, u, b]
        zd = zd.transpose(0, 3, 4, 5, 1, 2, 6, 7).reshape(NSB, 2, 64, 1024)

        brp = _pixelize(br_t, n)      # [sb, s, j, e, c, p]
        bip = _pixelize(bi_t, n)
        Y = np.stack([brp, bip], axis=4)   # [sb, s, j, e, u, c, p]
        # -> [sb, j, p, s, e, u, c]
        bt = Y.transpose(0, 2, 6, 1, 3, 4, 5).reshape(NSB, 128, 2048)

        in_maps.append({
            "wc": np.ascontiguousarray(wc),
            "zd": np.ascontiguousarray(zd),
            "bt": np.ascontiguousarray(bt),
        })
    return in_maps


def _assemble_output(res_list):
    out = np.empty((B, P, H, W), np.complex64)
    for n in range(NCORES):
        # part q = 32j + p ; free f = s*32 + e*16 + u*8 + b
        arr = res_list[n].reshape(NSB, 4, 32, NS, 2, 2, 8)
        # [sb, j, p, s, e, u, b] -> [b, p, sb, s, j, e, u]
        arr = arr.transpose(6, 2, 0, 3, 1, 4, 5)
        arr = np.ascontiguousarray(arr).reshape(B, P, HPER, W, 2)
        out[:, :, HPER * n:HPER * (n + 1), :] = arr[..., 0] + 1j * arr[..., 1]
    return out


_NC_CACHE = [None]


def _build_bass():
    if _NC_CACHE[0] is not None:
        return _NC_CACHE[0]
    import concourse.mybir as mybir
    import concourse.tile as tile
    from concourse import bacc

    f32 = mybir.dt.float32
    dt = getattr(mybir.dt, _DT_BASS)
    nc = bacc.Bacc("TRN2", target_bir_lowering=False, debug=False)
    wc_d = nc.dram_tensor("wc", [NSB, 128, 2048], dt, kind="ExternalInput")
    zd_d = nc.dram_tensor("zd", [NSB, 2, 64, 1024], dt, kind="ExternalInput")
    bt_d = nc.dram_tensor("bt", [NSB, 128, 2048], dt, kind="ExternalInput")
    out_d = nc.dram_tensor("out_dev", [NSB, 128, 512], f32,
                           kind="ExternalOutput")

    # Persistent triple-buffered z tensors (fixed SBUF allocations, outside
    # the Tile pools so their slots can't be recycled).
    zb_handles = [
        nc.alloc_sbuf_tensor(f"zb{i}", [128, ZFREE], dt) for i in range(3)
    ]

    with tile.TileContext(nc) as tc:
        with (
            tc.tile_pool(name="wp", bufs=4) as wpool,
            tc.tile_pool(name="bp", bufs=4) as bpool,
            tc.tile_pool(name="op", bufs=4) as opool,
            tc.tile_pool(name="pp", bufs=8, space="PSUM") as pspool,
        ):
            # block-diagonal rhs zero blocks, written once per z buffer.
            # zb free layout: [0:1024) = px0 data (rows 0-63) / zeros
            # (rows 64-127); [1024:2048) = zeros (rows 0-63) / px1 data
            # (rows 64-127). Keeps both the DMAs and the zero regions
            # contiguous; the matmul rhs uses a 2-level strided free AP.
            zbufs = []
            for i in range(3):
                zb = zb_handles[i][:]
                nc.vector.memset(zb[0:64, 1024:2048], 0.0)
                nc.vector.memset(zb[64:128, 0:1024], 0.0)
                zbufs.append(zb)

            for sb in range(NSB):
                wc_t = wpool.tile([128, 2048], dt, name="wc_t", tag="wc_t")
                nc.sync.dma_start(out=wc_t, in_=wc_d[sb])
                zb = zbufs[sb % 3]
                nc.sync.dma_start(out=zb[0:64, 0:1024], in_=zd_d[sb, 0])
                nc.sync.dma_start(out=zb[64:128, 1024:2048],
                                  in_=zd_d[sb, 1])
                zev = zb.rearrange("r (e f) -> r e f", e=2)
                bt_t = bpool.tile([128, 2048], dt, name="bt_t", tag="bt_t")
                nc.sync.dma_start(out=bt_t, in_=bt_d[sb])

                # bias-channel reduction on DVE: [128,(s,e,u),c32] -> [128,64]
                bs_t = bpool.tile([128, 64], f32, name="bs_t", tag="bs_t")
                nc.vector.reduce_sum(
                    bs_t, bt_t.rearrange("q (f c) -> q f c", c=32),
                    axis=mybir.AxisListType.X)

                # Full-bank psum tile (2048B/partition) so accumulation
                # zero-regions align exactly with 32-partition col-group
                # slices.
                ps = pspool.tile([128, 512], f32, name="ps", tag="ps")
                for s in range(NS):
                    # one K=128 matmul per pixel pair; consecutive matmuls
                    # hit different col-groups and run concurrently
                    for j in range(4):
                        pr = s * 4 + j
                        lhs = wc_t[:, pr * 32:pr * 32 + 32]
                        rhs = zev[:, :, pr * 16:pr * 16 + 16]
                        po = ps[32 * j:32 * j + 32, s * 32:s * 32 + 32]
                        nc.tensor.matmul(
                            po, lhs, rhs, start=True, stop=True,
                            tile_position=(0, 32 * j),
                            # sim's global group-check shadow mis-addresses
                            # partition-sliced psum outputs; the per-tensor
                            # pending-zero numerics are still modeled
                            # faithfully
                            skip_group_check=True,
                        )

                # evacuate psum + add bias in one pass per b-column
                o_t = opool.tile([128, 512], f32, name="o_t", tag="o_t")
                ov = o_t.rearrange("q (f b) -> q f b", b=8)
                pv = ps.rearrange("q (f b) -> q f b", b=8)
                for b8 in range(8):
                    nc.vector.tensor_add(ov[:, :, b8], pv[:, :, b8], bs_t)
                # output DMA on the ACT HWDGE ring: its compute-completion
                # wait must not head-of-line-block the input DMAs queued on
                # the SP ring
                nc.scalar.dma_start(out=out_d[sb], in_=o_t)

    nc.compile()
    _NC_CACHE[0] = nc
    return nc


def run(z, filt, bias, trace=False, trace_kwargs=None):
    """Returns (out, BassKernelResults)."""
    from concourse.bass_utils import run_bass_kernel_spmd
    in_maps = _prepare_inputs(z, filt, bias)
    nc = _build_bass()
    bkr = run_bass_kernel_spmd(
        nc, in_maps, core_ids=list(range(NCORES)),
        trace=trace, **(trace_kwargs or {}),
    )
    out = _assemble_output([r["out_dev"] for r in bkr.results])
    return out, bkr


def kernel(z, filt, bias):
    out, _ = run(np.asarray(z), np.asarray(filt), np.asarray(bias))
    return out
